# revision 54
# baseline (speedup 1.0000x reference)
"""Trainium2 Bass kernel for nn_Attribution (sparse local-window attention).

Data-parallel over batch n=8 -> one batch element per NeuronCore.

Per-core computation (c_in=256, ch=128, 64x64 image):
    h    = W1 @ x + b1
    corr = 5x5 local window correlation of h (zero padded), /sqrt(128)
    attn = softmax over the 25 window entries
    samp = sum_k attn_k * shift_k(h)
    gate = sigmoid(relu(W2 @ h + b2)) = 0.5 + 0.5*relu(tanh((z+b2)/2))
    out  = Wout @ (gate * samp) + bout

v2 schedule (single fused pipeline, one pass over 34 key chunks):
  front: conv1 + PE chunk transposes + conv2/gate, all interleaved per
         512-col block as x streams in (evacs split DVE/ACT).
  P3a:   per chunk pair: scores MM -> exp (ACT) -> band-mask (GpSimd)
         -> sample MMs of the previous pair (chunk-major, PSUM-accum
         per query sub) -> per-group denominator MMs -> short recip
         chain (one fused reciprocal_approx_fast DVE op on the (1,512)
         row) -> recip broadcast MM -> attr = sp*Pg*recip -> convout
         (bias via DVE tensor_scalar / ACT Identity+bias) -> bf16 out
         DMA.  Per-group chains keep TensorE dense (no 6us serial
         Newton stall, no HAM re-throttle).
"""
import sys

sys.path.insert(0, "/opt/trn_rl_repo")

import numpy as np
import ml_dtypes

import concourse.bass as bass
import concourse.mybir as mybir
import concourse.tile as tile
from concourse import bacc
from concourse.bass_utils import run_bass_kernel_spmd

F32 = mybir.dt.float32
BF16 = mybir.dt.bfloat16
AF = mybir.ActivationFunctionType
ALU = mybir.AluOpType

N, CIN, CH, H, W = 8, 256, 128, 64, 64
HW = H * W                      # 4096
RAD = 2
KROWS = H + 2 * RAD             # 68 padded rows
PADPOS = KROWS * W              # 4352
NCHUNK = PADPOS // 128          # 34 key chunks (2 rows each)
NSUB = H // 2                   # 32 query subs (128 queries each)
NGRP = NSUB // 4                # 8 groups of 4 subs (512 queries)
SCALE = 1.0 / np.sqrt(np.float32(CH))


def _build_mask_and_D():
    """maskC: (128, 384) {0,1}; col 128*a+q is the score of key (chunk c,
    pos p) vs query q of sub s = c-2+a.  Valid iff |2-2a + p//64 - q//64|
    <= 2 and |p%64 - q%64| <= 2.   dvrow: (512,) = 2*5*cnt(qx) tiled."""
    m = np.zeros((128, 384), dtype=np.float32)
    for a in range(3):
        for p in range(128):
            for q in range(128):
                dy = 2 - 2 * a + p // 64 - q // 64
                if abs(dy) <= RAD and abs(p % 64 - q % 64) <= RAD:
                    m[p, 128 * a + q] = 1.0
    cnt = np.array([sum(1 for dx in range(-RAD, RAD + 1) if not 0 <= qx + dx < W)
                    for qx in range(W)], dtype=np.float32)
    Drow = 5.0 * np.concatenate([cnt, cnt])          # (128,)
    dvrow = 2.0 * np.tile(Drow, 4)                   # (512,)
    return m, dvrow


def _span(c):
    lo, hi = max(0, c - 2), min(NSUB - 1, c)
    alo = lo - (c - 2)
    return lo, hi, alo, hi - lo + 1


def build_nc(repeat=1, sim_safe=False, mask_engine="gpsimd"):
    nc = bacc.Bacc("TRN2", target_bir_lowering=False, debug=False, num_devices=8)

    x_d = nc.declare_dram_parameter("x", [CIN, HW], BF16, isOutput=False)
    wfirst_d = nc.declare_dram_parameter("wfirst", [128, 384], BF16, isOutput=False)
    wsecond_d = nc.declare_dram_parameter("wsecond", [128, 513], BF16, isOutput=False)
    fbias_d = nc.declare_dram_parameter("fbias", [128, 4], F32, isOutput=False)
    drow_d = nc.declare_dram_parameter("drowp", [1, 512], BF16, isOutput=False)
    mblob_d = nc.declare_dram_parameter("mblob", [128, 384], BF16, isOutput=False)
    out_d = nc.declare_dram_parameter("out", [CIN, HW], BF16, isOutput=True)

    with tile.TileContext(nc) as tc:
        with (
            tc.tile_pool(name="per", bufs=1) as per,
            tc.tile_pool(name="smp", bufs=2) as smp,
            tc.tile_pool(name="otp", bufs=4) as otp,
            tc.tile_pool(name="psc", bufs=2, space="PSUM") as psc,   # score pairs
            tc.tile_pool(name="spp", bufs=2, space="PSUM") as spp,   # sample groups
            tc.tile_pool(name="pss", bufs=2, space="PSUM") as pss,   # generic 1-bank
        ):
            xa = per.tile([128, HW], BF16, tag="xa")
            xb = per.tile([128, HW], BF16, tag="xb")
            hpad = per.tile([128, PADPOS], BF16, tag="hpad")
            hT = per.tile([128, PADPOS], BF16, tag="hT")
            attnm = per.tile([128, NCHUNK * 512], BF16, tag="attnm")
            Pg = per.tile([128, HW], BF16, tag="Pg")
            attr = per.tile([128, HW], BF16, tag="attr")
            den2row = per.tile([1, HW], F32, tag="den2row")
            recf = per.tile([1, HW], F32, tag="recf")
            recrow = per.tile([1, HW], BF16, tag="recrow")
            denq = per.tile([128, 128], F32, tag="denq")
            newt = per.tile([128, 128], F32, tag="newt")
            denqb = per.tile([128, 128], BF16, tag="denqb")

            wfirst = per.tile([128, 384], BF16, tag="wfirst")
            wsecond = per.tile([128, 513], BF16, tag="wsecond")
            fbias = per.tile([128, 4], F32, tag="fbias")
            drowt = per.tile([1, 512], BF16, tag="drowt")
            mblob = per.tile([128, 384], BF16, tag="mblob")

            w1t0 = wfirst[:, 0:128]
            w1t1 = wfirst[:, 128:256]
            ident = wfirst[:, 256:384]
            w2t = wsecond[:, 0:128]
            wot0 = wsecond[:, 128:256]
            wot1 = wsecond[:, 256:384]
            onescol = wsecond[:, 384:385]
            onesrow = wsecond[0:1, 385:513]
            b1c = fbias[:, 0:1]
            b2hc = fbias[:, 1:2]
            bout0c = fbias[:, 2:3]
            bout1c = fbias[:, 3:4]
            one11 = wsecond[0:1, 384:385]   # (1,1) ones for the D rank-1 MM
            drow512 = drowt[0:1, 0:512]     # (1,512) D row const
            maskcomp = mblob[:, 0:384]      # -340 * (1 - mask), bf16

            for _rep in range(repeat):
                # ---- DMAs: graded x tiles (small first for a fast start,
                # big later for low descriptor-issue cost), consts just-in-time
                nc.sync.dma_start(wfirst[:], wfirst_d[:])
                nc.scalar.dma_start(fbias[:], fbias_d[:])
                xtiles = [(0, 512), (512, 1024), (1024, 2048), (2048, 4096)]
                for t, (lo, hi) in enumerate(xtiles):
                    cs = slice(lo, hi)
                    nc.sync.dma_start(xa[:, cs], x_d[0:128, cs])
                    nc.scalar.dma_start(xb[:, cs], x_d[128:256, cs])
                    if t == 0:
                        nc.sync.dma_start(mblob[:], mblob_d[:])
                    if t == 1:
                        nc.scalar.dma_start(wsecond[:], wsecond_d[:])
                    if t == 2:
                        nc.scalar.dma_start(drowt[:], drow_d[:])

                nc.vector.memset(hpad[:, 0:128], 0.0)
                nc.vector.memset(hpad[:, PADPOS - 128:PADPOS], 0.0)
                nc.vector.memset(hT[:, 0:128], 0.0)
                nc.vector.memset(hT[:, PADPOS - 128:PADPOS], 0.0)

                # PE-transposes don't count as "busy" for the HAM clock
                # gate, so they're dripped through the body (2 per pair,
                # just in time for the sample MMs) instead of bunching in
                # the front where they kept the PE at half clock.
                def emit_transposes(chunks):
                    pt = pss.tile([128, 512], BF16, tag="g",
                                  name=f"tp{chunks[0]}")
                    for k, c in enumerate(chunks):
                        nc.tensor.transpose(pt[:, 128 * k:128 * (k + 1)],
                                            hpad[:, 128 * c:128 * (c + 1)],
                                            ident)
                    tsl = slice(128 * chunks[0], 128 * (chunks[-1] + 1))
                    nc.scalar.activation(hT[:, tsl],
                                         pt[:, 0:128 * len(chunks)], AF.Copy)

                # ---- front block: conv1 + conv2/gate per 512-col block
                def front_block(u):
                    hsl = slice(128 + 512 * u, 128 + 512 * (u + 1))
                    ps = pss.tile([128, 512], F32, tag="g", name=f"cv1_{u}")
                    nc.tensor.matmul(ps[:], w1t0, xa[:, 512 * u:512 * (u + 1)],
                                     start=True, stop=False)
                    nc.tensor.matmul(ps[:], w1t1, xb[:, 512 * u:512 * (u + 1)],
                                     start=False, stop=True)
                    # ACT evac: its backlog at block boundaries (~0.5us) is
                    # much shallower than DVE's (chain ops, ~1.6us)
                    nc.scalar.activation(hpad[:, hsl], ps[:], AF.Identity,
                                         bias=b1c)
                    if u == 0:
                        emit_transposes([1, 2, 3, 4])
                    pz = pss.tile([128, 512], F32, tag="g", name=f"cv2_{u}")
                    nc.tensor.matmul(pz[:], w2t, hpad[:, hsl], start=True, stop=True)
                    tg = smp.tile([128, 512], BF16, tag="tg")
                    nc.scalar.activation(tg[:], pz[:], AF.Tanh, scale=0.5, bias=b2hc)
                    nc.vector.tensor_scalar(out=Pg[:, 512 * u:512 * (u + 1)],
                                            in0=tg[:], scalar1=0.0, scalar2=1.0,
                                            op0=ALU.max, op1=ALU.add)

                # ---- fused pair pipeline (17 chunk pairs)
                sp_tiles = {}

                def emit_samples_sub(s):
                    # sample MMs for query sub s (3 sequential accumulating
                    # MMs -> one psum group; groups in a bank never overlap)
                    g = s // 4
                    if s % 4 == 0:
                        sp_tiles[g] = spp.tile([128, 512], F32, tag="sp",
                                               name=f"sp{g}")
                    sp = sp_tiles[g]
                    for j in range(3):
                        c = s + j
                        aa = 2 - j
                        nc.tensor.matmul(
                            sp[:, 128 * (s % 4):128 * (s % 4 + 1)],
                            hT[:, 128 * c:128 * (c + 1)],
                            attnm[:, 512 * c + 128 * aa:512 * c + 128 * (aa + 1)],
                            start=(j == 0), stop=(j == 2))

                def emit_dn_chain(g):
                    # dn = D (rank-1 seed MM) + sum of masked exp; then
                    # recip on the (1,512) row and 0.5x into bf16 recrow.
                    dn = pss.tile([128, 512], F32, tag="g", name=f"dn{g}")
                    nc.tensor.matmul(dn[0:1, 0:512], one11, drow512,
                                     start=True, stop=False)
                    for k in range(4):
                        s = 4 * g + k
                        for j in range(3):
                            c = s + j
                            aa = 2 - j
                            nc.tensor.matmul(
                                dn[0:1, 128 * k:128 * (k + 1)], onescol,
                                attnm[:, 512 * c + 128 * aa:512 * c + 128 * (aa + 1)],
                                start=False, stop=(k == 3 and j == 2))
                    gs = slice(512 * g, 512 * (g + 1))
                    nc.vector.reciprocal_approx_fast(out=recf[0:1, gs],
                                                     in_=dn[0:1, 0:512])
                    nc.vector.tensor_scalar(out=recrow[0:1, gs],
                                            in0=recf[0:1, gs], scalar1=0.5,
                                            scalar2=None, op0=ALU.mult)

                def emit_attr1(g):
                    gs = slice(512 * g, 512 * (g + 1))
                    sp = sp_tiles.pop(g)
                    nc.vector.tensor_tensor(out=attr[:, gs], in0=sp[:],
                                            in1=Pg[:, gs], op=ALU.mult)

                def emit_pb_attr2(g):
                    gs = slice(512 * g, 512 * (g + 1))
                    pb = pss.tile([128, 512], F32, tag="g", name=f"pb{g}")
                    nc.tensor.matmul(pb[:], onesrow, recrow[0:1, gs],
                                     start=True, stop=True)
                    nc.vector.tensor_tensor(out=attr[:, gs], in0=attr[:, gs],
                                            in1=pb[:], op=ALU.mult)

                def emit_convout(g):
                    gs = slice(512 * g, 512 * (g + 1))
                    po0 = pss.tile([128, 512], F32, tag="g", name=f"po0_{g}")
                    nc.tensor.matmul(po0[:], wot0, attr[:, gs], start=True, stop=True)
                    ot0 = otp.tile([128, 512], BF16, tag="ot")
                    if g >= 6:
                        # tail: DVE is backlogged with chain(7)/attr work
                        nc.scalar.activation(ot0[:], po0[:], AF.Identity,
                                             bias=bout0c)
                    else:
                        nc.vector.tensor_scalar(out=ot0[:], in0=po0[:],
                                                scalar1=bout0c, scalar2=None,
                                                op0=ALU.add)
                    nc.sync.dma_start(out_d[0:128, gs], ot0[:])
                    po1 = pss.tile([128, 512], F32, tag="g", name=f"po1_{g}")
                    nc.tensor.matmul(po1[:], wot1, attr[:, gs], start=True, stop=True)
                    ot1 = otp.tile([128, 512], BF16, tag="ot")
                    nc.scalar.activation(ot1[:], po1[:], AF.Identity, bias=bout1c)
                    nc.scalar.dma_start(out_d[128:256, gs], ot1[:])

                def emit_scores_exp_mask(cp):
                    # scores + accumulated -340*(1-mask) bias (via identity
                    # stationary), then exp: masked entries become ~1e-13.
                    sc = psc.tile([128, 1024], F32, tag="sc", name=f"sc{cp}")
                    spans = []
                    for ci in range(2):
                        c = 2 * cp + ci
                        lo, hi, alo, n = _span(c)
                        spans.append((c, alo, n))
                        dst = sc[:, 512 * ci + 128 * alo:512 * ci + 128 * (alo + n)]
                        nc.tensor.matmul(
                            dst, hpad[:, 128 * c:128 * (c + 1)],
                            hpad[:, 128 * (lo + 1):128 * (hi + 2)],
                            start=True, stop=False)
                        nc.tensor.matmul(
                            dst, ident, maskcomp[:, 128 * alo:128 * (alo + n)],
                            start=False, stop=True)
                    if not sim_safe and 1 <= cp <= 15:
                        asl = attnm[:, 1024 * cp:1024 * cp + 896]
                        nc.scalar.activation(asl, sc[:, 0:896], AF.Exp,
                                             scale=float(SCALE))
                    else:
                        for ci, (c, alo, n) in enumerate(spans):
                            ss = slice(512 * ci + 128 * alo, 512 * ci + 128 * (alo + n))
                            asl = attnm[:, 512 * c + 128 * alo:
                                        512 * c + 128 * (alo + n)]
                            nc.scalar.activation(asl, sc[:, ss], AF.Exp,
                                                 scale=float(SCALE))

                # sub s's samples need chunks s..s+2 masked -> emit subs
                # {2cp-2, 2cp-1} at pair cp.  group g (subs 4g..4g+3) is
                # complete at cp=2g+2: dn+chain there, pb+attr2 at 2g+4
                # (emitted before the next chain so convout never waits
                # behind DVE chain work), convout at 2g+5.
                def pair_work(cp):
                    emit_scores_exp_mask(cp)
                    if cp >= 1:
                        emit_samples_sub(2 * cp - 2)
                        emit_samples_sub(2 * cp - 1)
                        tc_chunks = [c for c in (2 * cp + 3, 2 * cp + 4)
                                     if 1 <= c <= 32]
                        if tc_chunks:
                            emit_transposes(tc_chunks)
                    if cp >= 4 and cp % 2 == 0:
                        emit_pb_attr2((cp - 4) // 2)
                    if cp >= 2 and cp % 2 == 0:
                        g = (cp - 2) // 2
                        emit_dn_chain(g)
                        emit_attr1(g)
                    if cp >= 5 and cp % 2 == 1:
                        emit_convout((cp - 5) // 2)

                # interleave: after front block u, pairs up to 2u are
                # eligible (scores pair cp reads hpad chunks <= 2cp+3).
                done = 0
                for u in range(8):
                    front_block(u)
                    while done <= min(2 * u, 16):
                        pair_work(done)
                        done += 1
                while done <= 16:
                    pair_work(done)
                    done += 1

                # ---- tail: groups 6/7 wrap-up (pb/attr2(6) ran at cp=16)
                emit_convout(6)
                emit_pb_attr2(7)
                emit_convout(7)

    return nc


def _prep_inputs(x, W1, b1, W2, b2, Wout, bout):
    maskC_np, dvrow = _build_mask_and_D()
    bf = ml_dtypes.bfloat16

    wfirst = np.zeros((128, 384), np.float32)
    W1T = np.ascontiguousarray(np.asarray(W1, np.float32).T)     # (256, 128)
    wfirst[:, 0:128] = W1T[0:128]
    wfirst[:, 128:256] = W1T[128:256]
    wfirst[:, 256:384] = np.eye(128, dtype=np.float32)

    wsecond = np.zeros((128, 513), np.float32)
    wsecond[:, 0:128] = np.asarray(W2, np.float32).T
    WoutT = np.ascontiguousarray(np.asarray(Wout, np.float32).T)  # (128, 256)
    wsecond[:, 128:384] = WoutT
    wsecond[:, 384] = 1.0
    wsecond[0, 385:513] = 1.0

    fbias = np.zeros((128, 4), np.float32)
    fbias[:, 0] = np.asarray(b1, np.float32)
    fbias[:, 1] = 0.5 * np.asarray(b2, np.float32)
    fbias[:, 2] = np.asarray(bout, np.float32)[0:128]
    fbias[:, 3] = np.asarray(bout, np.float32)[128:256]

    mblob = -340.0 * (1.0 - maskC_np)        # (128, 384)

    common = {
        "wfirst": wfirst.astype(bf),
        "wsecond": wsecond.astype(bf),
        "fbias": fbias,
        "drowp": (0.5 * dvrow).reshape(1, 512).astype(bf),   # plain D row
        "mblob": mblob.astype(bf),
    }
    in_maps = []
    for i in range(N):
        m = dict(common)
        m["x"] = np.ascontiguousarray(
            np.asarray(x[i], np.float32).reshape(CIN, HW)).astype(bf)
        in_maps.append(m)
    return in_maps


_CACHED = {}


def kernel(x, W1, b1, W2, b2, Wout, bout):
    if "nc" not in _CACHED:
        nc = build_nc()
        nc.finalize()
        _CACHED["nc"] = nc
    nc = _CACHED["nc"]
    in_maps = _prep_inputs(x, W1, b1, W2, b2, Wout, bout)
    res = run_bass_kernel_spmd(nc, in_maps, core_ids=list(range(N)))
    out = np.stack([np.asarray(res.results[i]["out"], np.float32).reshape(CIN, H, W)
                    for i in range(N)])
    return out.astype(np.float32)
